# revision 1
# baseline (speedup 1.0000x reference)
"""CARAFE + MSGConv Trainium2 kernel (8 NeuronCores, spatial x batch sharding).

out[c, i, j] = sum_{p,q} W[5p+q, i, j] * Xpad[c, i//2 + p - 2, j//2 + q - 2]
 (CARAFE taps live at source resolution; identical for both subpixel parities).

Per core: one batch element (core//4) and a 16-source-row block (core%4).
The 25-tap reassembly runs on the TensorEngine as one K=120 matmul per
(row-pair, column-quarter) block:
  out[c, n] = sum_{(u,v)} X6T[(u,v), c] * B4[(u,v), n]
where B4 is a banded matrix of softmaxed W values built at runtime with
gpsimd local_scatter (per-partition index scatter) + a PE transpose; the
X side (X6T) is static data and comes pre-transposed from the host.
"""

import sys

sys.path.insert(0, "/opt/trn_rl_repo")

from contextlib import ExitStack

import ml_dtypes
import numpy as np

import concourse.bass as bass
import concourse.tile as tile
from concourse import bacc, library_config, mybir
from concourse.bass_utils import run_bass_kernel_spmd

BF16 = mybir.dt.bfloat16
F32 = mybir.dt.float32
I16 = mybir.dt.int16
AF = mybir.ActivationFunctionType
OP = mybir.AluOpType
nbf = ml_dtypes.bfloat16

C = 128
H = W = 64
NCORES = 8
XR = 24          # X shard rows (16 + 4 halo each side)
XW = 68          # padded width for dw slabs only
NEG = -30.0      # additive pre-activation mask; SiLU(-30) ~= -2.8e-12


# ======================================================================
# host-side parameter prep
# ======================================================================

def _fold_1x1(w, s):
    return (w[:, :, 0, 0] * s[:, None]).T.copy()


def _dw_taps(w, s, k):
    ch = w.shape[0]
    out = np.zeros((ch, 25), np.float32)
    off = (5 - k) // 2
    for ty in range(k):
        for tx in range(k):
            out[:, 5 * (ty + off) + (tx + off)] = w[:, 0, ty, tx] * s
    return out


def _host_consts(inputs):
    d = {}
    w_cv1 = _fold_1x1(inputs["comp_cv1_w"], inputs["comp_cv1_s"])
    b_cv1 = inputs["comp_cv1_b"].reshape(32, 1)
    w3 = _dw_taps(inputs["comp_dw3_w"], inputs["comp_dw3_s"], 3)
    w5 = _dw_taps(inputs["comp_dw5_w"], inputs["comp_dw5_s"], 5)
    w_dwp = np.tile(np.concatenate([w3, w5], 0), (4, 1))
    b_dwp = np.tile(
        np.concatenate([inputs["comp_dw3_b"], inputs["comp_dw5_b"]]), 4
    ).reshape(128, 1)
    w_px = _fold_1x1(inputs["comp_px_w"], inputs["comp_px_s"])
    b_px = inputs["comp_px_b"].reshape(64, 1)
    we = _fold_1x1(inputs["enc_cv1_w"], inputs["enc_cv1_s"])
    w_ecv1 = np.concatenate([we, np.ones((1, 50), np.float32)], 0)
    b_ecv1 = inputs["enc_cv1_b"].reshape(50, 1)
    e3 = _dw_taps(inputs["enc_dw3_w"], inputs["enc_dw3_s"], 3)
    e5 = _dw_taps(inputs["enc_dw5_w"], inputs["enc_dw5_s"], 5)
    w_edwp = np.tile(np.concatenate([e3, e5], 0), (2, 1))
    b_edwp = np.tile(
        np.concatenate([inputs["enc_dw3_b"], inputs["enc_dw5_b"]]), 2
    ).reshape(100, 1)
    wpx = _fold_1x1(inputs["enc_px_w"], inputs["enc_px_s"])
    w_epx = np.concatenate([wpx, inputs["enc_px_b"].reshape(1, 100)], 0)

    # packA bf16 [128, 374]: w_cv1 | w_px | w_ecv1 | w_epx | ident
    pa = np.zeros((128, 374), np.float32)
    pa[0:128, 0:32] = w_cv1
    pa[0:64, 32:96] = w_px
    pa[0:65, 96:146] = w_ecv1
    pa[0:101, 146:246] = w_epx
    pa[0:128, 246:374] = np.eye(128)
    d["packa"] = pa.astype(nbf)
    # packB f32 [128, 55]
    pb = np.zeros((128, 55), np.float32)
    pb[:, 0:25] = w_dwp
    pb[:, 25:26] = b_dwp
    pb[0:100, 26:51] = w_edwp
    pb[0:100, 51:52] = b_edwp
    pb[0:32, 52:53] = b_cv1
    pb[0:64, 53:54] = b_px
    pb[0:50, 54:55] = b_ecv1
    d["packb"] = pb

    d["ones1"] = np.ones((1, 32), nbf)
    d["erow1"] = np.ones((1, 16 * W), nbf)

    # repl [128, 4*128]: lhsT for the W row-replication matmul
    # n raster within a block: n = 32*(2*yl+dy) + (2*xl+dx)
    rp = np.zeros((128, 512), np.float32)
    for jb in range(4):
        for n in range(128):
            rho, j = divmod(n, 32)
            yl, xl = rho // 2, j // 2
            rp[64 * yl + 16 * jb + xl, 128 * jb + n] = 1.0
    d["repl"] = rp.astype(nbf)

    # sidx [128, 4*100] int16 (4 blocks = one row-pair per scatter call);
    # horizontal out-of-image taps are dropped here (-1 = skipped).
    si = np.full((128, 400), -1, np.int16)
    for n in range(128):
        rho, j = divmod(n, 32)
        yl, dy = divmod(rho, 2)
        xl, dx = divmod(j, 2)
        sn = 2 * dy + dx
        for jb in range(4):
            for cp in range(100):
                sc, k = divmod(cp, 25)
                if sc != sn:
                    continue
                p, q = divmod(k, 5)
                if not (0 <= 16 * jb + xl + q - 2 < 64):
                    continue
                si[n, 100 * jb + cp] = 120 * jb + 20 * (yl + p) + (xl + q)
    d["sidx"] = si
    return d


def _host_shard(X, core):
    b, ri = divmod(core, 4)
    r0 = 16 * ri - 4
    xs = np.zeros((C, XR, W), np.float32)
    lo, hi = max(0, r0), min(H, r0 + XR)
    xs[:, lo - r0 : hi - r0, :] = X[b, :, lo:hi, :]
    mrow = np.zeros((1, XR, W), np.float32)
    for r in range(XR):
        if not (0 <= r0 + r < H):
            mrow[0, r, :] = NEG
    emask = np.zeros((1, 20, W), np.float32)
    for r in range(20):
        if not (0 <= (16 * ri - 2) + r < H):
            emask[0, r, :] = NEG
    xsb = xs.astype(nbf)
    # pre-transposed X slabs, one [120, 128] per block (column-padded)
    xsp = np.zeros((C, XR, XW), nbf)
    xsp[:, :, 2 : 2 + W] = xsb
    xt = np.zeros((120, 32 * 128), nbf)
    for B in range(32):
        t, jb = divmod(B, 4)
        slab = xsp[:, 2 * t + 2 : 2 * t + 8, 16 * jb : 16 * jb + 20]
        xt[:, 128 * B : 128 * B + 128] = slab.reshape(C, 120).T
    return (
        xsb.reshape(C, XR * W),
        mrow.reshape(1, XR * W).astype(nbf),
        emask.reshape(1, 20 * W).astype(nbf),
        xt,
    )


# ======================================================================
# device kernel
# ======================================================================

def build_kernel():
    nc = bacc.Bacc(
        "TRN2",
        target_bir_lowering=False,
        debug=False,
        enable_asserts=False,
        num_devices=NCORES,
    )

    def din(name, shape, dt):
        return nc.dram_tensor(name, list(shape), dt, kind="ExternalInput").ap()

    x_d = din("x", (128, XR * W), BF16)
    xt_d = din("xt", (120, 32 * 128), BF16)
    mrow_d = din("mrow", (1, XR * W), BF16)
    emask_d = din("emask", (1, 20 * W), BF16)
    erow1_d = din("erow1", (1, 16 * W), BF16)
    ones1_d = din("ones1", (1, 32), BF16)
    packa_d = din("packa", (128, 374), BF16)
    packb_d = din("packb", (128, 55), F32)
    repl_d = din("repl", (128, 512), BF16)
    sidx_d = din("sidx", (128, 400), I16)
    out_d = nc.dram_tensor("out", [128, 32 * 128], F32, kind="ExternalOutput").ap()
    out3 = out_d.rearrange("c (r j) -> c r j", j=128)

    with tile.TileContext(nc) as tc, ExitStack() as ctx:
        cpool = ctx.enter_context(tc.tile_pool(name="consts", bufs=1))
        work = ctx.enter_context(tc.tile_pool(name="work", bufs=1))
        psB = ctx.enter_context(tc.tile_pool(name="psB", bufs=2, space="PSUM"))
        psC = ctx.enter_context(tc.tile_pool(name="psC", bufs=3, space="PSUM"))
        spool = ctx.enter_context(tc.tile_pool(name="stage", bufs=3))
        psA_cm = tc.tile_pool(name="psA", bufs=2, space="PSUM")
        psA = psA_cm.__enter__()

        nc.gpsimd.load_library(library_config.local_scatter)

        def cload(ap_d, shape, dt, eng=None):
            t = cpool.tile(list(shape), dt, tag=ap_d.tensor.name)
            (eng or nc.sync).dma_start(t[:], ap_d)
            return t

        packa = cload(packa_d, (128, 374), BF16)
        packb = cload(packb_d, (128, 55), F32)
        mrow = cload(mrow_d, (1, XR * W), BF16, eng=nc.scalar)
        ones1 = cload(ones1_d, (1, 32), BF16, eng=nc.scalar)
        w_cv1 = packa[0:128, 0:32]
        w_px = packa[0:64, 32:96]
        w_ecv1 = packa[0:65, 96:146]
        w_epx = packa[0:101, 146:246]
        ident = packa[0:128, 246:374]
        w_dwp = packb[0:128, 0:25]
        b_dwp = packb[0:128, 25:26]
        w_edwp = packb[0:100, 26:51]
        b_edwp = packb[0:100, 51:52]
        b_cv1 = packb[0:32, 52:53]
        b_px = packb[0:64, 53:54]
        b_ecv1 = packb[0:50, 54:55]
        xb = cpool.tile([128, XR * W], BF16, tag="x")
        for ch in range(3):
            (nc.sync if ch != 1 else nc.scalar).dma_start(
                xb[:, 8 * W * ch : 8 * W * (ch + 1)],
                x_d[:, 8 * W * ch : 8 * W * (ch + 1)],
            )
        xt = cpool.tile([120, 32 * 128], BF16, tag="xt")
        for ch in range(2):
            nc.gpsimd.dma_start(
                xt[:, 2048 * ch : 2048 * (ch + 1)],
                xt_d[:, 2048 * ch : 2048 * (ch + 1)],
            )
        repl = cload(repl_d, (128, 512), BF16, eng=nc.gpsimd)
        sidx = cload(sidx_d, (128, 400), I16, eng=nc.gpsimd)

        # warmup: trigger the local_scatter ucode library load early so it
        # overlaps the conv front instead of stalling the first real scatter
        warm = work.tile([16, 16], BF16)
        nc.gpsimd.local_scatter(
            warm[:], packa[0:16, 0:2], sidx[:][0:16, 0:2],
            channels=16, num_elems=16, num_idxs=2,
        )

        # persistent working tensors (all 64-wide / contiguous)
        x12 = work.tile([64, XR * W], BF16)        # x1 (0:32) + x2 (32:64)
        enc_in = work.tile([65, 20 * W], BF16)     # px out + mask row
        enc_cat = work.tile([101, 16 * W], BF16)   # enc x1/x2 + ones row
        e1c = work.tile([50, 20 * W], BF16)        # enc cv1 out
        x1p = work.tile([128, 9 * XW + 8], BF16)   # packed x1 (68-pitch)
        e1p = work.tile([100, 12 * XW + 8], BF16)  # packed enc x1 (68-pitch)
        ET = work.tile([128, 800], F32)
        expv = work.tile([128, 800], F32)          # [s][t][k]
        S = work.tile([128, 32], F32)
        R = work.tile([128, 32], F32)
        wcat = work.tile([128, 800], BF16)         # [t][s][k]
        dall = work.tile([128, 3200], BF16)
        b4t = work.tile([128, 4 * 960], BF16)

        xb3 = xb[:].rearrange("p (r c) -> p r c", c=W)
        mrow3 = mrow[:].rearrange("p (r c) -> p r c", c=W)
        x12_3 = x12[:].rearrange("p (r c) -> p r c", c=W)
        enc_in3 = enc_in[:].rearrange("p (r c) -> p r c", c=W)
        enc_cat3 = enc_cat[:].rearrange("p (r c) -> p r c", c=W)
        e1c3 = e1c[:].rearrange("p (r c) -> p r c", c=W)
        x1p3 = x1p[:, 0 : 9 * XW].rearrange("p (r c) -> p r c", c=XW)
        e1p3 = e1p[:, 0 : 12 * XW].rearrange("p (r c) -> p r c", c=XW)
        ET3 = ET[:].rearrange("p (t e) -> p t e", e=100)
        exp3 = expv[:].rearrange("p (s t k) -> p s t k", s=4, t=8)

        # zero only the dw-slab pad columns (cols 0:2 and 66:68)
        nc.vector.memset(x1p[:, 9 * XW : 9 * XW + 8], 0.0)
        nc.vector.memset(e1p[:, 12 * XW : 12 * XW + 8], 0.0)
        nc.vector.memset(x1p3[:, :, 0:2], 0.0)
        nc.vector.memset(x1p3[:, :, 66:68], 0.0)
        nc.vector.memset(e1p3[:, :, 0:2], 0.0)
        nc.vector.memset(e1p3[:, :, 66:68], 0.0)
        nc.sync.dma_start(enc_cat[100:101, :], erow1_d)
        nc.sync.dma_start(enc_in[64:65, :], emask_d)

        # ---- comp cv1: 1x1 conv 128->32 (+ SiLU + out-of-image row mask)
        for ch in range(3):
            ps = psA.tile([32, 512], F32, tag="convps")
            nc.tensor.matmul(
                ps[:], w_cv1, xb[:, 512 * ch : 512 * (ch + 1)],
                start=True, stop=False,
            )
            nc.tensor.matmul(
                ps[:], ones1[:], mrow[:, 512 * ch : 512 * (ch + 1)],
                start=False, stop=True,
            )
            nc.scalar.activation(
                x12[0:32, 512 * ch : 512 * (ch + 1)], ps[:],
                AF.Silu, bias=b_cv1,
            )

        # ---- comp dw3/dw5 (unified 5x5 taps, rows packed 4x32)
        for g in range(4):
            nc.sync.dma_start(
                x1p3[32 * g : 32 * g + 32, 0:9, 2 : 2 + W],
                x12_3[0:32, 5 * g : 5 * g + 9, :],
            )
        FS = 5 * XW                    # 340
        acc_a = work.tile([128, FS], BF16)
        acc_b = work.tile([128, FS], BF16)
        accv = [acc_a[:], acc_b[:]]
        for t in range(25):
            ty, tx = divmod(t, 5)
            sv = x1p[:, ty * XW + tx : ty * XW + tx + FS]
            av = accv[t % 2]
            if t < 2:
                nc.vector.tensor_scalar(av, sv, w_dwp[:, t : t + 1], None, OP.mult)
            else:
                nc.vector.scalar_tensor_tensor(
                    av, sv, w_dwp[:, t : t + 1], av, OP.mult, OP.add
                )
        nc.vector.tensor_add(accv[0], accv[0], accv[1])
        x2p = work.tile([128, FS], BF16)
        nc.scalar.activation(x2p[:], acc_a[:], AF.Silu, bias=b_dwp)
        for g in range(4):
            nc.sync.dma_start(
                x12_3[32:64, 2 + 5 * g : 7 + 5 * g, :],
                x2p[32 * g : 32 * g + 32, :].rearrange(
                    "p (r c) -> p r c", c=XW
                )[:, 0:5, 0:W],
            )

        # ---- comp px: 1x1 conv 64->64 (+ SiLU)
        for r0, nr in ((0, 8), (8, 8), (16, 4)):
            ps = psA.tile([64, 512], F32, tag="convps")
            nc.tensor.matmul(
                ps[:, : nr * W], w_px,
                x12[0:64, (2 + r0) * W : (2 + r0 + nr) * W],
                start=True, stop=True,
            )
            nc.scalar.activation(
                enc_in[0:64, r0 * W : (r0 + nr) * W], ps[:, : nr * W],
                AF.Silu, bias=b_px,
            )

        # ---- enc cv1: 1x1 conv 64->50 (+ SiLU, mask row rides K=65)
        for r0, nr in ((0, 8), (8, 8), (16, 4)):
            ps = psA.tile([50, 512], F32, tag="convps")
            nc.tensor.matmul(
                ps[:, : nr * W], w_ecv1,
                enc_in[0:65, r0 * W : (r0 + nr) * W],
                start=True, stop=True,
            )
            nc.scalar.activation(
                e1c[0:50, r0 * W : (r0 + nr) * W], ps[:, : nr * W],
                AF.Silu, bias=b_ecv1,
            )

        # ---- enc dw3/dw5 (rows packed 2x50)
        for g in range(2):
            nc.sync.dma_start(
                e1p3[50 * g : 50 * g + 50, 0:12, 2 : 2 + W],
                e1c3[0:50, 8 * g : 8 * g + 12, :],
            )
        nc.sync.dma_start(enc_cat[0:50, :], e1c[0:50, 2 * W : 18 * W])
        FS2 = 8 * XW                   # 544
        acc2_a = work.tile([100, FS2], BF16)
        acc2_b = work.tile([100, FS2], BF16)
        acc2v = [acc2_a[:], acc2_b[:]]
        for t in range(25):
            ty, tx = divmod(t, 5)
            sv = e1p[:, ty * XW + tx : ty * XW + tx + FS2]
            av = acc2v[t % 2]
            if t < 2:
                nc.vector.tensor_scalar(av, sv, w_edwp[:, t : t + 1], None, OP.mult)
            else:
                nc.vector.scalar_tensor_tensor(
                    av, sv, w_edwp[:, t : t + 1], av, OP.mult, OP.add
                )
        nc.vector.tensor_add(acc2v[0], acc2v[0], acc2v[1])
        e2p = work.tile([100, FS2], BF16)
        nc.scalar.activation(e2p[:], acc2_a[:], AF.Silu, bias=b_edwp)
        for g in range(2):
            nc.sync.dma_start(
                enc_cat3[50:100, 8 * g : 8 * g + 8, :],
                e2p[50 * g : 50 * g + 50, :].rearrange(
                    "p (r c) -> p r c", c=XW
                )[:, 0:8, 0:W],
            )

        # ---- enc px (transposed output: M = 128 pixels per row-pair)
        for t in range(8):
            ps = psA.tile([128, 100], F32, tag="convps")
            nc.tensor.matmul(
                ps[:], enc_cat[0:101, 128 * t : 128 * t + 128],
                w_epx, start=True, stop=True,
            )
            nc.scalar.activation(ET[:, 100 * t : 100 * t + 100], ps[:], AF.Silu)

        # ---- softmax over 25 taps per subposition (no max-subtraction)
        for s in range(4):
            nc.scalar.activation(exp3[:, s], ET3[:, :, s::4], AF.Exp)
            nc.vector.tensor_reduce(
                S[:, 8 * s : 8 * s + 8], exp3[:, s], mybir.AxisListType.X, OP.add
            )
        nc.vector.reciprocal(R[:], S[:])
        psA_cm.__exit__(None, None, None)
        psO = ctx.enter_context(tc.tile_pool(name="psO", bufs=3, space="PSUM"))
        stgs = []
        for t in range(8):
            for s in range(4):
                dstw = wcat[:, 100 * t + 25 * s : 100 * t + 25 * s + 25]
                if (4 * t + s) % 2 == 0:
                    nc.vector.tensor_scalar(
                        dstw, exp3[:, s, t],
                        R[:, 8 * s + t : 8 * s + t + 1], None, OP.mult,
                    )
                else:
                    nc.scalar.activation(
                        dstw, exp3[:, s, t], AF.Copy,
                        scale=R[:, 8 * s + t : 8 * s + t + 1],
                    )
            for jb in range(4):
                B = 4 * t + jb
                ps = psB.tile([128, 100], F32, tag="small")
                nc.tensor.matmul(
                    ps[:], repl[:, 128 * jb : 128 * jb + 128],
                    wcat[:, 100 * t : 100 * t + 100],
                    start=True, stop=True,
                )
                dst = dall[:, 100 * B : 100 * B + 100]
                if B % 2 == 0:
                    nc.vector.tensor_copy(dst, ps[:])
                else:
                    nc.scalar.copy(dst, ps[:])
            nc.gpsimd.local_scatter(
                b4t[:, 480 * t : 480 * t + 480],
                dall[:, 400 * t : 400 * t + 400],
                sidx[:],
                channels=128, num_elems=480, num_idxs=400,
            )

        for B in range(32):
            t, jb = divmod(B, 4)
            psb4 = psC.tile([120, 128], BF16, tag="b4t")
            nc.tensor.transpose(
                psb4[:], b4t[:, 480 * t + 120 * jb : 480 * t + 120 * jb + 120],
                ident,
            )
            b4 = spool.tile([120, 128], BF16, tag="b4")
            if B % 2 == 0:
                nc.scalar.copy(b4[:], psb4[:])
            else:
                nc.vector.tensor_copy(b4[:], psb4[:])

            po = psO.tile([128, 128], F32, tag="out")
            nc.tensor.matmul(
                po[:], xt[:, 128 * B : 128 * B + 128], b4[:],
                start=True, stop=True,
            )

            if jb == 0:
                stg = spool.tile([128, 512], F32, tag="ostage")
                stgs.append(stg)
            stg = stgs[-1]
            sv_dst = stg[:].rearrange("c (r j) -> c r j", j=128)[
                :, :, 32 * jb : 32 * jb + 32
            ]
            if B % 2 == 0:
                nc.vector.tensor_copy(
                    sv_dst, po[:].rearrange("c (r j) -> c r j", j=32)
                )
            else:
                nc.scalar.copy(
                    sv_dst, po[:].rearrange("c (r j) -> c r j", j=32)
                )
            if jb == 3:
                (nc.sync if t % 2 == 0 else nc.scalar).dma_start(
                    out3[:, 4 * t : 4 * t + 4, :],
                    stg[:].rearrange("c (r j) -> c r j", j=128),
                )

    nc.compile()
    return nc


_NC_CACHE = None


def _get_nc():
    global _NC_CACHE
    if _NC_CACHE is None:
        _NC_CACHE = build_kernel()
    return _NC_CACHE


def kernel(**inputs) -> np.ndarray:
    X = np.asarray(inputs["X"], np.float32)
    consts = _host_consts(
        {k: np.asarray(v, np.float32) for k, v in inputs.items() if k != "X"}
    )
    in_maps = []
    for core in range(NCORES):
        xs, mrow, emask, xt = _host_shard(X, core)
        m = dict(consts)
        m["x"] = xs
        m["mrow"] = mrow
        m["emask"] = emask
        m["xt"] = xt
        in_maps.append(m)

    nc = _get_nc()
    res = run_bass_kernel_spmd(nc, in_maps, core_ids=list(range(NCORES)))
    out = np.zeros((2, C, 128, 128), np.float32)
    for core in range(NCORES):
        b, ri = divmod(core, 4)
        out[b, :, 32 * ri : 32 * ri + 32, :] = (
            res.results[core]["out"].reshape(C, 32, 128)
        )
    return out


if __name__ == "__main__":
    print("smoke build only")
    build_kernel()
    print("build ok")



# revision 9
# speedup vs baseline: 1.1717x; 1.1717x over previous
"""CARAFE + MSGConv Trainium2 kernel (8 NeuronCores, spatial x batch sharding).

out[c, i, j] = sum_{p,q} W[5p+q, i, j] * Xpad[c, i//2 + p - 2, j//2 + q - 2]
 (CARAFE taps live at source resolution; identical for both subpixel parities).

Per core: one batch element (core//4) and a 16-source-row block (core%4).

v2 design:
- Depthwise convs run on the TensorEngine as 25 accumulating diagonal
  matmuls per conv (lhsT = diag(w[:,t]) shipped from host), alternating
  two PSUM banks so weight loads pipeline with streaming.
- The 25-tap reassembly weights are scattered in PIXEL space (100 idx per
  row-pair instead of 400 in output space: no quadrant replication), then
  4 per-subpixel-plane PE transposes write an interleaved SBUF layout so
  each output block's matmul rhs is fully contiguous.
- Softmax runs per row-pair (exp -> strided reduce -> recip -> one
  broadcast tensor_tensor), pipelined with the scatter/transpose/matmul
  back end.
"""

import sys

sys.path.insert(0, "/opt/trn_rl_repo")

from contextlib import ExitStack

import ml_dtypes
import numpy as np

import concourse.bass as bass
import concourse.tile as tile
from concourse import bacc, library_config, mybir
from concourse.bass_utils import run_bass_kernel_spmd

BF16 = mybir.dt.bfloat16
F32 = mybir.dt.float32
I16 = mybir.dt.int16
AF = mybir.ActivationFunctionType
OP = mybir.AluOpType
nbf = ml_dtypes.bfloat16

C = 128
H = W = 64
NCORES = 8
XR = 24          # X shard rows (16 + 4 halo each side)
XW = 68          # padded width for dw slabs only
NEG = -30.0      # additive pre-activation mask; SiLU(-30) ~= -2.8e-12


# ======================================================================
# host-side parameter prep
# ======================================================================

def _fold_1x1(w, s):
    return (w[:, :, 0, 0] * s[:, None]).T.copy()


def _dw_taps(w, s, k):
    ch = w.shape[0]
    out = np.zeros((ch, 25), np.float32)
    off = (5 - k) // 2
    for ty in range(k):
        for tx in range(k):
            out[:, 5 * (ty + off) + (tx + off)] = w[:, 0, ty, tx] * s
    return out


def _host_consts(inputs):
    d = {}
    w_cv1 = _fold_1x1(inputs["comp_cv1_w"], inputs["comp_cv1_s"])
    b_cv1 = inputs["comp_cv1_b"]
    w3 = _dw_taps(inputs["comp_dw3_w"], inputs["comp_dw3_s"], 3)
    w5 = _dw_taps(inputs["comp_dw5_w"], inputs["comp_dw5_s"], 5)
    w_dwp = np.tile(np.concatenate([w3, w5], 0), (4, 1))      # [128, 25]
    b_dwp = np.tile(
        np.concatenate([inputs["comp_dw3_b"], inputs["comp_dw5_b"]]), 4
    )
    w_px = _fold_1x1(inputs["comp_px_w"], inputs["comp_px_s"])
    b_px = inputs["comp_px_b"]
    we = _fold_1x1(inputs["enc_cv1_w"], inputs["enc_cv1_s"])
    w_ecv1 = np.concatenate([we, np.ones((1, 50), np.float32)], 0)
    b_ecv1 = inputs["enc_cv1_b"]
    e3 = _dw_taps(inputs["enc_dw3_w"], inputs["enc_dw3_s"], 3)
    e5 = _dw_taps(inputs["enc_dw5_w"], inputs["enc_dw5_s"], 5)
    w_edwp = np.tile(np.concatenate([e3, e5], 0), (2, 1))     # [100, 25]
    b_edwp = np.tile(
        np.concatenate([inputs["enc_dw3_b"], inputs["enc_dw5_b"]]), 2
    )
    wpx = _fold_1x1(inputs["enc_px_w"], inputs["enc_px_s"])
    w_epx = np.concatenate([wpx, inputs["enc_px_b"].reshape(1, 100)], 0)

    # packa bf16 [128, 374]: w_cv1 | w_px | w_ecv1 | w_epx | ident
    pa = np.zeros((128, 374), np.float32)
    pa[0:128, 0:32] = w_cv1
    pa[0:64, 32:96] = w_px
    pa[0:65, 96:146] = w_ecv1
    pa[0:101, 146:246] = w_epx
    pa[0:128, 246:374] = np.eye(128)
    d["packa"] = pa.astype(nbf)
    # packb f32 [128, 5]: biases only
    pb = np.zeros((128, 5), np.float32)
    pb[:, 0] = b_dwp
    pb[0:100, 1] = b_edwp
    pb[0:32, 2] = b_cv1
    pb[0:64, 3] = b_px
    pb[0:50, 4] = b_ecv1
    d["packb"] = pb

    # diagonal tap matrices for the PE depthwise convs
    dc = np.zeros((128, 25, 128), np.float32)
    for t in range(25):
        np.fill_diagonal(dc[:, t, :], w_dwp[:, t])
    d["diagc"] = dc.reshape(128, 3200).astype(nbf)
    de = np.zeros((100, 25, 100), np.float32)
    for t in range(25):
        np.fill_diagonal(de[:, t, :], w_edwp[:, t])
    d["diage"] = de.reshape(100, 2500).astype(nbf)

    d["ones1"] = np.ones((1, 32), nbf)
    d["erow1"] = np.ones((1, 16 * W), nbf)

    # sidx [128, 100] int16: pixel-space scatter, pix = 64*yl + x
    # entry e = 4k+s -> 120*s + 20*(yl+p) + (x%16 + q), k = 5p+q
    si = np.zeros((128, 100), np.int16)
    for pix in range(128):
        yl, x = divmod(pix, 64)
        xl = x % 16
        for k in range(25):
            p, q = divmod(k, 5)
            for s in range(4):
                si[pix, 4 * k + s] = 120 * s + 20 * (yl + p) + (xl + q)
    d["sidx"] = si
    return d


def _host_shard(X, core):
    b, ri = divmod(core, 4)
    r0 = 16 * ri - 4
    xs = np.zeros((C, XR, W), np.float32)
    lo, hi = max(0, r0), min(H, r0 + XR)
    xs[:, lo - r0 : hi - r0, :] = X[b, :, lo:hi, :]
    mrow = np.zeros((1, XR, W), np.float32)
    for r in range(XR):
        if not (0 <= r0 + r < H):
            mrow[0, r, :] = NEG
    emask = np.zeros((1, 20, W), np.float32)
    for r in range(20):
        if not (0 <= (16 * ri - 2) + r < H):
            emask[0, r, :] = NEG
    xsb = xs.astype(nbf)
    # pre-transposed X slabs, one [120, 128] per block (column-padded)
    xsp = np.zeros((C, XR, XW), nbf)
    xsp[:, :, 2 : 2 + W] = xsb
    xt = np.zeros((120, 32 * 128), nbf)
    for B in range(32):
        t, jb = divmod(B, 4)
        slab = xsp[:, 2 * t + 2 : 2 * t + 8, 16 * jb : 16 * jb + 20]
        xt[:, 128 * B : 128 * B + 128] = slab.reshape(C, 120).T
    return (
        xsb.reshape(C, XR * W),
        mrow.reshape(1, XR * W).astype(nbf),
        emask.reshape(1, 20 * W).astype(nbf),
        xt,
    )


# ======================================================================
# device kernel
# ======================================================================

def build_kernel():
    nc = bacc.Bacc(
        "TRN2",
        target_bir_lowering=False,
        debug=False,
        enable_asserts=False,
        num_devices=NCORES,
    )

    def din(name, shape, dt):
        return nc.dram_tensor(name, list(shape), dt, kind="ExternalInput").ap()

    x_d = din("x", (128, XR * W), BF16)
    xt_d = din("xt", (120, 32 * 128), BF16)
    mrow_d = din("mrow", (1, XR * W), BF16)
    emask_d = din("emask", (1, 20 * W), BF16)
    erow1_d = din("erow1", (1, 16 * W), BF16)
    ones1_d = din("ones1", (1, 32), BF16)
    packa_d = din("packa", (128, 374), BF16)
    packb_d = din("packb", (128, 5), F32)
    diagc_d = din("diagc", (128, 3200), BF16)
    diage_d = din("diage", (100, 2500), BF16)
    sidx_d = din("sidx", (128, 100), I16)
    out_d = nc.dram_tensor("out", [128, 32 * 128], F32, kind="ExternalOutput").ap()
    out3 = out_d.rearrange("c (r j) -> c r j", j=128)

    with tile.TileContext(nc) as tc, ExitStack() as ctx:
        cpool = ctx.enter_context(tc.tile_pool(name="consts", bufs=1))
        work = ctx.enter_context(tc.tile_pool(name="work", bufs=1))
        spool = ctx.enter_context(tc.tile_pool(name="stage", bufs=2))
        psA_cm = tc.tile_pool(name="psA", bufs=2, space="PSUM")
        psA = psA_cm.__enter__()
        psDW_cm = tc.tile_pool(name="psDW", bufs=1, space="PSUM")
        psDW = psDW_cm.__enter__()

        nc.gpsimd.load_library(library_config.local_scatter)

        def cload(ap_d, shape, dt, eng=None):
            t = cpool.tile(list(shape), dt, tag=ap_d.tensor.name)
            (eng or nc.sync).dma_start(t[:], ap_d)
            return t

        packa = cload(packa_d, (128, 374), BF16)
        packb = cload(packb_d, (128, 5), F32)
        mrow = cload(mrow_d, (1, XR * W), BF16, eng=nc.scalar)
        ones1 = cload(ones1_d, (1, 32), BF16, eng=nc.scalar)
        w_cv1 = packa[0:128, 0:32]
        w_px = packa[0:64, 32:96]
        w_ecv1 = packa[0:65, 96:146]
        w_epx = packa[0:101, 146:246]
        ident = packa[0:128, 246:374]
        b_dwp = packb[0:128, 0:1]
        b_edwp = packb[0:100, 1:2]
        b_cv1 = packb[0:32, 2:3]
        b_px = packb[0:64, 3:4]
        b_ecv1 = packb[0:50, 4:5]
        xb = cpool.tile([128, XR * W], BF16, tag="x")
        for ch in range(3):
            (nc.sync if ch != 1 else nc.scalar).dma_start(
                xb[:, 8 * W * ch : 8 * W * (ch + 1)],
                x_d[:, 8 * W * ch : 8 * W * (ch + 1)],
            )
        sidx = cload(sidx_d, (128, 100), I16, eng=nc.gpsimd)
        diagc = cload(diagc_d, (128, 3200), BF16, eng=nc.gpsimd)
        diage = cload(diage_d, (100, 2500), BF16, eng=nc.gpsimd)
        xt = cpool.tile([120, 32 * 128], BF16, tag="xt")
        for ch in range(2):
            nc.gpsimd.dma_start(
                xt[:, 2048 * ch : 2048 * (ch + 1)],
                xt_d[:, 2048 * ch : 2048 * (ch + 1)],
            )

        # warmup: trigger the local_scatter ucode library load early
        warm = work.tile([16, 16], BF16)
        nc.gpsimd.local_scatter(
            warm[:], packa[0:16, 0:2], sidx[:][0:16, 0:2],
            channels=16, num_elems=16, num_idxs=2,
        )

        # persistent working tensors
        x12 = work.tile([64, XR * W], BF16)        # x1 (0:32) + x2 (32:64)
        enc_in = work.tile([65, 20 * W], BF16)     # px out + mask row
        enc_cat = work.tile([101, 16 * W], BF16)   # enc x1/x2 + ones row
        e1c = work.tile([50, 20 * W], BF16)        # enc cv1 out
        x1p = work.tile([128, 9 * XW + 8], BF16)   # packed x1 (68-pitch)
        e1p = work.tile([100, 12 * XW + 8], BF16)  # packed enc x1 (68-pitch)
        ET = work.tile([128, 800], F32)            # enc px logits
        expb = work.tile([128, 800], BF16)         # exp values [t][4k+s]
        S = work.tile([128, 32], F32)
        R = work.tile([128, 32], F32)
        wcats = work.tile([128, 800], BF16)        # softmaxed weights
        b4x = work.tile([128, 8 * 480], BF16)      # pix-space bands
        x2p = work.tile([128, 340], BF16)
        e2p = work.tile([100, 544], BF16)

        xb3 = xb[:].rearrange("p (r c) -> p r c", c=W)
        x12_3 = x12[:].rearrange("p (r c) -> p r c", c=W)
        enc_cat3 = enc_cat[:].rearrange("p (r c) -> p r c", c=W)
        e1c3 = e1c[:].rearrange("p (r c) -> p r c", c=W)
        x1p3 = x1p[:, 0 : 9 * XW].rearrange("p (r c) -> p r c", c=XW)
        e1p3 = e1p[:, 0 : 12 * XW].rearrange("p (r c) -> p r c", c=XW)
        ET3 = ET[:].rearrange("p (t e) -> p t e", e=100)
        Rv = R[:].rearrange("p (t s) -> p t s", s=4)
        Sv = S[:].rearrange("p (t s) -> p t s", s=4)

        # zero the dw-slab pad columns (cols 0:2 and 66:68)
        nc.vector.memset(x1p[:, 9 * XW : 9 * XW + 8], 0.0)
        nc.vector.memset(e1p[:, 12 * XW : 12 * XW + 8], 0.0)
        nc.vector.memset(x1p3[:, :, 0:2], 0.0)
        nc.vector.memset(x1p3[:, :, 66:68], 0.0)
        nc.vector.memset(e1p3[:, :, 0:2], 0.0)
        nc.vector.memset(e1p3[:, :, 66:68], 0.0)
        nc.sync.dma_start(enc_cat[100:101, :], erow1_d)
        nc.sync.dma_start(enc_in[64:65, :], emask_d)

        # ---- comp cv1: 1x1 conv 128->32 (+ SiLU + out-of-image row mask)
        for ch in range(3):
            ps = psA.tile([32, 512], F32, tag="convps")
            nc.tensor.matmul(
                ps[:], w_cv1, xb[:, 512 * ch : 512 * (ch + 1)],
                start=True, stop=False,
            )
            nc.tensor.matmul(
                ps[:], ones1[:], mrow[:, 512 * ch : 512 * (ch + 1)],
                start=False, stop=True,
            )
            nc.scalar.activation(
                x12[0:32, 512 * ch : 512 * (ch + 1)], ps[:],
                AF.Silu, bias=b_cv1,
            )

        # ---- comp dw3/dw5: 25 diag matmuls, 2 PSUM chunks of 170
        for g in range(4):
            nc.sync.dma_start(
                x1p3[32 * g : 32 * g + 32, 0:9, 2 : 2 + W],
                x12_3[0:32, 5 * g : 5 * g + 9, :],
            )
        ps_c0 = psDW.tile([128, 170], F32, tag="dwc0")
        ps_c1 = psDW.tile([128, 170], F32, tag="dwc1")
        for t in range(25):
            ty, tx = divmod(t, 5)
            off = ty * XW + tx
            lhsT = diagc[:, 128 * t : 128 * t + 128]
            nc.tensor.matmul(ps_c0[:], lhsT, x1p[:, off : off + 170],
                             start=(t == 0), stop=(t == 24))
            nc.tensor.matmul(ps_c1[:], lhsT, x1p[:, off + 170 : off + 340],
                             start=(t == 0), stop=(t == 24))
        nc.scalar.activation(x2p[:, 0:170], ps_c0[:], AF.Silu, bias=b_dwp)
        nc.scalar.activation(x2p[:, 170:340], ps_c1[:], AF.Silu, bias=b_dwp)
        for g in range(4):
            nc.sync.dma_start(
                x12_3[32:64, 2 + 5 * g : 7 + 5 * g, :],
                x2p[32 * g : 32 * g + 32, :].rearrange(
                    "p (r c) -> p r c", c=XW
                )[:, 0:5, 0:W],
            )

        # ---- comp px: 1x1 conv 64->64 (+ SiLU)
        for r0, nr in ((0, 8), (8, 8), (16, 4)):
            ps = psA.tile([64, 512], F32, tag="convps")
            nc.tensor.matmul(
                ps[:, : nr * W], w_px,
                x12[0:64, (2 + r0) * W : (2 + r0 + nr) * W],
                start=True, stop=True,
            )
            nc.scalar.activation(
                enc_in[0:64, r0 * W : (r0 + nr) * W], ps[:, : nr * W],
                AF.Silu, bias=b_px,
            )

        # ---- enc cv1: 1x1 conv 64->50 (+ SiLU, mask row rides K=65)
        for r0, nr in ((0, 8), (8, 8), (16, 4)):
            ps = psA.tile([50, 512], F32, tag="convps")
            nc.tensor.matmul(
                ps[:, : nr * W], w_ecv1,
                enc_in[0:65, r0 * W : (r0 + nr) * W],
                start=True, stop=True,
            )
            nc.scalar.activation(
                e1c[0:50, r0 * W : (r0 + nr) * W], ps[:, : nr * W],
                AF.Silu, bias=b_ecv1,
            )

        # ---- enc dw3/dw5: 25 diag matmuls, 2 PSUM chunks of 272
        for g in range(2):
            nc.sync.dma_start(
                e1p3[50 * g : 50 * g + 50, 0:12, 2 : 2 + W],
                e1c3[0:50, 8 * g : 8 * g + 12, :],
            )
        nc.sync.dma_start(enc_cat[0:50, :], e1c[0:50, 2 * W : 18 * W])
        ps_e0 = psDW.tile([100, 272], F32, tag="dwe0")
        ps_e1 = psDW.tile([100, 272], F32, tag="dwe1")
        for t in range(25):
            ty, tx = divmod(t, 5)
            off = ty * XW + tx
            lhsT = diage[:, 100 * t : 100 * t + 100]
            nc.tensor.matmul(ps_e0[:], lhsT, e1p[0:100, off : off + 272],
                             start=(t == 0), stop=(t == 24))
            nc.tensor.matmul(ps_e1[:], lhsT, e1p[0:100, off + 272 : off + 544],
                             start=(t == 0), stop=(t == 24))
        nc.scalar.activation(e2p[:, 0:272], ps_e0[:], AF.Silu, bias=b_edwp)
        nc.scalar.activation(e2p[:, 272:544], ps_e1[:], AF.Silu, bias=b_edwp)
        for g in range(2):
            nc.sync.dma_start(
                enc_cat3[50:100, 8 * g : 8 * g + 8, :],
                e2p[50 * g : 50 * g + 50, :].rearrange(
                    "p (r c) -> p r c", c=XW
                )[:, 0:8, 0:W],
            )

        # ---- enc px (transposed output: M = 128 pixels per row-pair)
        for t in range(8):
            ps = psA.tile([128, 100], F32, tag="encpx")
            nc.tensor.matmul(
                ps[:], enc_cat[0:101, 128 * t : 128 * t + 128],
                w_epx, start=True, stop=True,
            )
            nc.scalar.activation(ET[:, 100 * t : 100 * t + 100], ps[:], AF.Silu)

        psDW_cm.__exit__(None, None, None)
        psA_cm.__exit__(None, None, None)
        psT = ctx.enter_context(tc.tile_pool(name="psT", bufs=3, space="PSUM"))
        psO = ctx.enter_context(tc.tile_pool(name="psO", bufs=2, space="PSUM"))

        # ---- per row-pair: softmax -> scatter -> transpose -> matmul
        for t in range(8):
            te = expb[:, 100 * t : 100 * t + 100]
            tw = wcats[:, 100 * t : 100 * t + 100]
            # exp (bf16), sum over k (stride-4), 1/S
            nc.scalar.activation(te, ET3[:, t, :], AF.Exp)
            nc.vector.tensor_reduce(
                Sv[:, t], te.rearrange("p (k s) -> p s k", s=4),
                mybir.AxisListType.X, OP.add,
            )
            nc.vector.reciprocal(Rv[:, t], Sv[:, t])
            # scale: exp * R, R broadcast over k
            rb = Rv[:, t].unsqueeze(1).broadcast_to([128, 25, 4])
            nc.vector.tensor_tensor(
                tw.rearrange("p (k s) -> p k s", s=4),
                te.rearrange("p (k s) -> p k s", s=4), rb, OP.mult,
            )
            # pixel-space band scatter
            bx = b4x[:, 480 * t : 480 * t + 480]
            nc.gpsimd.local_scatter(
                bx, tw, sidx[:], channels=128, num_elems=480, num_idxs=100,
            )
            # 4 plane transposes -> interleaved b4all
            b4all = spool.tile([120, 512], BF16, tag="b4all")
            bview = b4all[:].rearrange(
                "p (jb yl dy xl dx) -> p dy dx yl jb xl",
                jb=4, yl=2, dy=2, xl=16, dx=2,
            )
            for s in range(4):
                pst = psT.tile([120, 128], BF16, tag="tr")
                nc.tensor.transpose(pst[:], bx[:, 120 * s : 120 * s + 120],
                                    ident)
                src = pst[:].rearrange("p (yl jb xl) -> p yl jb xl",
                                       yl=2, jb=4)
                dst = bview[:, s // 2, s % 2]
                if s % 2 == 0:
                    nc.vector.tensor_copy(dst, src)
                else:
                    nc.scalar.copy(dst, src)
            # 4 output matmuls into one PSUM bank
            po = psO.tile([128, 512], F32, tag="out")
            for jb in range(4):
                nc.tensor.matmul(
                    po[:, 128 * jb : 128 * jb + 128],
                    xt[:, 512 * t + 128 * jb : 512 * t + 128 * jb + 128],
                    b4all[:, 128 * jb : 128 * jb + 128],
                    start=True, stop=True,
                )
            stg = spool.tile([128, 512], F32, tag="ostage")
            stg3 = stg[:].rearrange("c (r j) -> c r j", j=128)
            for jb in range(4):
                src = po[:, 128 * jb : 128 * jb + 128].rearrange(
                    "c (r j) -> c r j", j=32)
                dst = stg3[:, :, 32 * jb : 32 * jb + 32]
                if jb % 2 == 0:
                    nc.vector.tensor_copy(dst, src)
                else:
                    nc.scalar.copy(dst, src)
            (nc.sync if t % 2 == 0 else nc.scalar).dma_start(
                out3[:, 4 * t : 4 * t + 4, :],
                stg[:].rearrange("c (r j) -> c r j", j=128),
            )

    nc.compile()
    return nc


_NC_CACHE = None


def _get_nc():
    global _NC_CACHE
    if _NC_CACHE is None:
        _NC_CACHE = build_kernel()
    return _NC_CACHE


def kernel(**inputs) -> np.ndarray:
    X = np.asarray(inputs["X"], np.float32)
    consts = _host_consts(
        {k: np.asarray(v, np.float32) for k, v in inputs.items() if k != "X"}
    )
    in_maps = []
    for core in range(NCORES):
        xs, mrow, emask, xt = _host_shard(X, core)
        m = dict(consts)
        m["x"] = xs
        m["mrow"] = mrow
        m["emask"] = emask
        m["xt"] = xt
        in_maps.append(m)

    nc = _get_nc()
    res = run_bass_kernel_spmd(nc, in_maps, core_ids=list(range(NCORES)))
    out = np.zeros((2, C, 128, 128), np.float32)
    for core in range(NCORES):
        b, ri = divmod(core, 4)
        out[b, :, 32 * ri : 32 * ri + 32, :] = (
            res.results[core]["out"].reshape(C, 32, 128)
        )
    return out


if __name__ == "__main__":
    print("smoke build only")
    build_kernel()
    print("build ok")


# revision 34
# speedup vs baseline: 1.2339x; 1.0531x over previous
"""CARAFE + MSGConv Trainium2 kernel (8 NeuronCores, spatial x batch sharding).

out[c, i, j] = sum_{p,q} W[5p+q, i, j] * Xpad[c, i//2 + p - 2, j//2 + q - 2]
 (CARAFE taps live at source resolution; identical for both subpixel parities).

Per core: one batch element (core//4) and a 16-source-row block (core%4).

v2 design:
- Depthwise convs run on the TensorEngine as 25 accumulating diagonal
  matmuls per conv (lhsT = diag(w[:,t]) shipped from host), alternating
  two PSUM banks so weight loads pipeline with streaming.
- The 25-tap reassembly weights are scattered in PIXEL space (100 idx per
  row-pair instead of 400 in output space: no quadrant replication), then
  4 per-subpixel-plane PE transposes write an interleaved SBUF layout so
  each output block's matmul rhs is fully contiguous.
- Softmax runs per row-pair (exp -> strided reduce -> recip -> one
  broadcast tensor_tensor), pipelined with the scatter/transpose/matmul
  back end.
"""

import sys

sys.path.insert(0, "/opt/trn_rl_repo")

from contextlib import ExitStack

import ml_dtypes
import numpy as np

import concourse.bass as bass
import concourse.tile as tile
from concourse import bacc, library_config, mybir
from concourse.bass_utils import run_bass_kernel_spmd

BF16 = mybir.dt.bfloat16
F32 = mybir.dt.float32
I16 = mybir.dt.int16
AF = mybir.ActivationFunctionType
OP = mybir.AluOpType
nbf = ml_dtypes.bfloat16

C = 128
H = W = 64
NCORES = 8
XR = 24          # X shard rows (16 + 4 halo each side)
XW = 68          # padded width for dw slabs only
NEG = -30.0      # additive pre-activation mask; SiLU(-30) ~= -2.8e-12


# ======================================================================
# host-side parameter prep
# ======================================================================

def _fold_1x1(w, s):
    return (w[:, :, 0, 0] * s[:, None]).T.copy()


def _dw_taps(w, s, k):
    ch = w.shape[0]
    out = np.zeros((ch, 25), np.float32)
    off = (5 - k) // 2
    for ty in range(k):
        for tx in range(k):
            out[:, 5 * (ty + off) + (tx + off)] = w[:, 0, ty, tx] * s
    return out


def _host_consts(inputs):
    d = {}
    w_cv1 = _fold_1x1(inputs["comp_cv1_w"], inputs["comp_cv1_s"])
    b_cv1 = inputs["comp_cv1_b"]
    w3 = _dw_taps(inputs["comp_dw3_w"], inputs["comp_dw3_s"], 3)
    w5 = _dw_taps(inputs["comp_dw5_w"], inputs["comp_dw5_s"], 5)
    w_dwp = np.tile(np.concatenate([w3, w5], 0), (4, 1))      # [128, 25]
    b_dwp = np.tile(
        np.concatenate([inputs["comp_dw3_b"], inputs["comp_dw5_b"]]), 4
    )
    w_px = _fold_1x1(inputs["comp_px_w"], inputs["comp_px_s"])
    b_px = inputs["comp_px_b"]
    we = _fold_1x1(inputs["enc_cv1_w"], inputs["enc_cv1_s"])
    w_ecv1 = np.concatenate([we, np.ones((1, 50), np.float32)], 0)
    b_ecv1 = inputs["enc_cv1_b"]
    e3 = _dw_taps(inputs["enc_dw3_w"], inputs["enc_dw3_s"], 3)
    e5 = _dw_taps(inputs["enc_dw5_w"], inputs["enc_dw5_s"], 5)
    w_edwp = np.tile(np.concatenate([e3, e5], 0), (2, 1))     # [100, 25]
    b_edwp = np.tile(
        np.concatenate([inputs["enc_dw3_b"], inputs["enc_dw5_b"]]), 2
    )
    wpx = _fold_1x1(inputs["enc_px_w"], inputs["enc_px_s"])
    w_epx = np.concatenate([wpx, inputs["enc_px_b"].reshape(1, 100)], 0)

    # packa bf16 [128, 374]: w_cv1 | w_px | w_ecv1 | w_epx | ident
    pa = np.zeros((128, 374), np.float32)
    pa[0:128, 0:32] = w_cv1
    pa[0:64, 32:96] = w_px
    pa[0:65, 96:146] = w_ecv1
    pa[0:101, 146:246] = w_epx
    pa[0:128, 246:374] = np.eye(128)
    d["packa"] = pa.astype(nbf)
    # packb f32 [128, 5]: biases only
    pb = np.zeros((128, 5), np.float32)
    pb[:, 0] = b_dwp
    pb[0:100, 1] = b_edwp
    pb[0:32, 2] = b_cv1
    pb[0:64, 3] = b_px
    pb[0:50, 4] = b_ecv1
    d["packb"] = pb

    # depthwise tap weights (diag matrices are built on-device by scatter);
    # chunk layout: 12 taps + 14 taps (num_idxs must be even; the 14th
    # duplicates tap 24, writing the same diag slot twice)
    wt = np.zeros((128, 52), np.float32)
    wt[:, 0:12] = w_dwp[:, 0:12]
    wt[:, 12:25] = w_dwp[:, 12:25]
    wt[:, 25] = w_dwp[:, 24]
    wt[0:100, 26:38] = w_edwp[:, 0:12]
    wt[0:100, 38:51] = w_edwp[:, 12:25]
    wt[0:100, 51] = w_edwp[:, 24]
    d["wtap"] = wt.astype(nbf)
    # scatter indices for the diag build: col j -> diag block j, own row
    dg = np.zeros((128, 28), np.int16)
    for p in range(128):
        for j in range(14):
            dg[p, j] = 128 * min(j, 12) + p
            dg[p, 14 + j] = 100 * min(j, 12) + p if p < 100 else 0
    d["dgix"] = dg

    d["ones1"] = np.ones((1, 32), nbf)
    d["erow1"] = np.ones((1, 16 * W), nbf)

    # sidx [128, 100] int16: pixel-space scatter, pix = 64*yl + x
    # entry e = 4k+s -> 120*s + 20*(yl+p) + (x%16 + q), k = 5p+q
    si = np.zeros((128, 100), np.int16)
    for pix in range(128):
        yl, x = divmod(pix, 64)
        xl = x % 16
        for k in range(25):
            p, q = divmod(k, 5)
            for s in range(4):
                si[pix, 4 * k + s] = 120 * s + 20 * (yl + p) + (xl + q)
    d["sidx"] = si
    return d


def _host_shard(X, core):
    b, ri = divmod(core, 4)
    r0 = 16 * ri - 4
    xs = np.zeros((C, XR, W), np.float32)
    lo, hi = max(0, r0), min(H, r0 + XR)
    xs[:, lo - r0 : hi - r0, :] = X[b, :, lo:hi, :]
    mrow = np.zeros((1, XR, W), np.float32)
    for r in range(XR):
        if not (0 <= r0 + r < H):
            mrow[0, r, :] = NEG
    emask = np.zeros((1, 20, W), np.float32)
    for r in range(20):
        if not (0 <= (16 * ri - 2) + r < H):
            emask[0, r, :] = NEG
    xsb = xs.astype(nbf)
    # pre-transposed X slabs, one [120, 128] per block (column-padded)
    xsp = np.zeros((C, XR, XW), nbf)
    xsp[:, :, 2 : 2 + W] = xsb
    xt = np.zeros((120, 32 * 128), nbf)
    for B in range(32):
        t, jb = divmod(B, 4)
        slab = xsp[:, 2 * t + 2 : 2 * t + 8, 16 * jb : 16 * jb + 20]
        xt[:, 128 * B : 128 * B + 128] = slab.reshape(C, 120).T
    return (
        xsb.reshape(C, XR * W),
        mrow.reshape(1, XR * W).astype(nbf),
        emask.reshape(1, 20 * W).astype(nbf),
        xt,
    )


# ======================================================================
# device kernel
# ======================================================================

def build_kernel():
    nc = bacc.Bacc(
        "TRN2",
        target_bir_lowering=False,
        debug=False,
        enable_asserts=False,
        num_devices=NCORES,
    )

    def din(name, shape, dt):
        return nc.dram_tensor(name, list(shape), dt, kind="ExternalInput").ap()

    x_d = din("x", (128, XR * W), BF16)
    xt_d = din("xt", (120, 32 * 128), BF16)
    mrow_d = din("mrow", (1, XR * W), BF16)
    emask_d = din("emask", (1, 20 * W), BF16)
    erow1_d = din("erow1", (1, 16 * W), BF16)
    ones1_d = din("ones1", (1, 32), BF16)
    packa_d = din("packa", (128, 374), BF16)
    packb_d = din("packb", (128, 5), F32)
    wtap_d = din("wtap", (128, 52), BF16)
    dgix_d = din("dgix", (128, 28), I16)
    sidx_d = din("sidx", (128, 100), I16)
    out_d = nc.dram_tensor("out", [128, 32 * 128], F32, kind="ExternalOutput").ap()
    out3 = out_d.rearrange("c (r j) -> c r j", j=128)

    with tile.TileContext(nc) as tc, ExitStack() as ctx:
        cpool = ctx.enter_context(tc.tile_pool(name="consts", bufs=1))
        work = ctx.enter_context(tc.tile_pool(name="work", bufs=1))
        spool = ctx.enter_context(tc.tile_pool(name="stage", bufs=2))
        psA_cm = tc.tile_pool(name="psA", bufs=2, space="PSUM")
        psA = psA_cm.__enter__()
        psDW_cm = tc.tile_pool(name="psDW", bufs=1, space="PSUM")
        psDW = psDW_cm.__enter__()

        nc.gpsimd.load_library(library_config.local_scatter)

        def cload(ap_d, shape, dt, eng=None):
            t = cpool.tile(list(shape), dt, tag=ap_d.tensor.name)
            (eng or nc.sync).dma_start(t[:], ap_d)
            return t

        # sync queue: cv1 weights first, then the X chunks, then the rest
        packa = cpool.tile([128, 374], BF16, tag="packa")
        nc.sync.dma_start(packa[:, 0:32], packa_d[:, 0:32])
        packb = cload(packb_d, (128, 5), F32, eng=nc.scalar)
        mrow = cload(mrow_d, (1, XR * W), BF16, eng=nc.scalar)
        ones1 = cload(ones1_d, (1, 32), BF16, eng=nc.scalar)
        w_cv1 = packa[0:128, 0:32]
        w_px = packa[0:64, 32:96]
        w_ecv1 = packa[0:65, 96:146]
        w_epx = packa[0:101, 146:246]
        ident = packa[0:128, 246:374]
        b_dwp = packb[0:128, 0:1]
        b_edwp = packb[0:100, 1:2]
        b_cv1 = packb[0:32, 2:3]
        b_px = packb[0:64, 3:4]
        b_ecv1 = packb[0:50, 4:5]
        xb = cpool.tile([128, XR * W], BF16, tag="x")
        for ch in range(3):
            (nc.sync if ch != 1 else nc.scalar).dma_start(
                xb[:, 8 * W * ch : 8 * W * (ch + 1)],
                x_d[:, 8 * W * ch : 8 * W * (ch + 1)],
            )
        nc.sync.dma_start(packa[:, 32:374], packa_d[:, 32:374])
        wtap = cload(wtap_d, (128, 52), BF16, eng=nc.gpsimd)
        dgix = cload(dgix_d, (128, 28), I16, eng=nc.gpsimd)
        sidx = cload(sidx_d, (128, 100), I16, eng=nc.gpsimd)
        xt = cpool.tile([120, 32 * 128], BF16, tag="xt")
        for ch in range(2):
            nc.gpsimd.dma_start(
                xt[:, 2048 * ch : 2048 * (ch + 1)],
                xt_d[:, 2048 * ch : 2048 * (ch + 1)],
            )

        # warmup: trigger the local_scatter ucode library load early
        warm = work.tile([16, 16], BF16)
        nc.gpsimd.local_scatter(
            warm[:], packa[0:16, 0:2], dgix[:][0:16, 0:2],
            channels=16, num_elems=16, num_idxs=2,
        )

        # build the depthwise diag matrices on-device (scatter zero-fills)
        diagc = cpool.tile([128, 3200], BF16, tag="diagc")
        diage = cpool.tile([128, 2500], BF16, tag="diage")
        nc.gpsimd.local_scatter(
            diagc[:, 0:1536], wtap[:, 0:12], dgix[:, 0:12],
            channels=128, num_elems=1536, num_idxs=12,
        )
        nc.gpsimd.local_scatter(
            diagc[:, 1536:3200], wtap[:, 12:26], dgix[:, 0:14],
            channels=128, num_elems=1664, num_idxs=14,
        )
        nc.gpsimd.local_scatter(
            diage[:, 0:1200], wtap[:, 26:38], dgix[:, 14:26],
            channels=128, num_elems=1200, num_idxs=12,
        )
        nc.gpsimd.local_scatter(
            diage[:, 1200:2500], wtap[:, 38:52], dgix[:, 14:28],
            channels=128, num_elems=1300, num_idxs=14,
        )

        # persistent working tensors
        x12 = work.tile([64, XR * W], BF16)        # x1 (0:32) + x2 (32:64)
        enc_in = work.tile([65, 20 * W], BF16)     # px out + mask row
        enc_cat = work.tile([101, 16 * W], BF16)   # enc x1/x2 + ones row
        e1c = work.tile([50, 20 * W], BF16)        # enc cv1 out
        x1p = work.tile([128, 9 * XW + 8], BF16)   # packed x1 (68-pitch)
        e1p = work.tile([100, 12 * XW + 8], BF16)  # packed enc x1 (68-pitch)
        ET = work.tile([128, 800], F32)            # enc px logits
        expb = work.tile([128, 800], BF16)         # exp values [t][4k+s]
        S = work.tile([128, 32], F32)
        R = work.tile([128, 32], F32)
        wcats = work.tile([128, 800], BF16)        # softmaxed weights
        b4x = work.tile([128, 8 * 480], BF16)      # pix-space bands
        x2p = work.tile([128, 340], BF16)
        e2p = work.tile([100, 544], BF16)

        xb3 = xb[:].rearrange("p (r c) -> p r c", c=W)
        x12_3 = x12[:].rearrange("p (r c) -> p r c", c=W)
        enc_cat3 = enc_cat[:].rearrange("p (r c) -> p r c", c=W)
        e1c3 = e1c[:].rearrange("p (r c) -> p r c", c=W)
        x1p3 = x1p[:, 0 : 9 * XW].rearrange("p (r c) -> p r c", c=XW)
        e1p3 = e1p[:, 0 : 12 * XW].rearrange("p (r c) -> p r c", c=XW)
        ET3 = ET[:].rearrange("p (t e) -> p t e", e=100)
        Rv = R[:].rearrange("p (t s) -> p t s", s=4)
        Sv = S[:].rearrange("p (t s) -> p t s", s=4)

        # zero the dw-slab pad columns (cols 0:2 and 66:68)
        nc.vector.memset(x1p[:, 9 * XW : 9 * XW + 8], 0.0)
        nc.vector.memset(e1p[:, 12 * XW : 12 * XW + 8], 0.0)
        nc.vector.memset(x1p3[:, :, 0:2], 0.0)
        nc.vector.memset(x1p3[:, :, 66:68], 0.0)
        nc.vector.memset(e1p3[:, :, 0:2], 0.0)
        nc.vector.memset(e1p3[:, :, 66:68], 0.0)
        nc.sync.dma_start(enc_cat[100:101, :], erow1_d)
        nc.sync.dma_start(enc_in[64:65, :], emask_d)

        # ---- comp cv1: 1x1 conv 128->32 (+ SiLU + out-of-image row mask)
        for ch in range(3):
            ps = psA.tile([32, 512], F32, tag="convps")
            nc.tensor.matmul(
                ps[:], w_cv1, xb[:, 512 * ch : 512 * (ch + 1)],
                start=True, stop=False,
            )
            nc.tensor.matmul(
                ps[:], ones1[:], mrow[:, 512 * ch : 512 * (ch + 1)],
                start=False, stop=True,
            )
            nc.scalar.activation(
                x12[0:32, 512 * ch : 512 * (ch + 1)], ps[:],
                AF.Silu, bias=b_cv1,
            )

        # ---- comp dw3/dw5: 25 diag matmuls, 2 PSUM chunks of 170
        for g in range(4):
            nc.sync.dma_start(
                x1p3[32 * g : 32 * g + 32, 0:9, 2 : 2 + W],
                x12_3[0:32, 5 * g : 5 * g + 9, :],
            )
        # even taps accumulate in bank A, odd in bank B (pipelined LDW)
        ps_c0 = psDW.tile([128, 340], F32, tag="dwc0")
        ps_c1 = psDW.tile([128, 340], F32, tag="dwc1")
        for t in range(25):
            ty, tx = divmod(t, 5)
            off = ty * XW + tx
            nc.tensor.matmul((ps_c0 if t % 2 == 0 else ps_c1)[:],
                             diagc[:, 128 * t : 128 * t + 128],
                             x1p[:, off : off + 340],
                             start=(t < 2), stop=(t >= 23))
        tmpb = work.tile([128, 340], BF16)
        accd = work.tile([128, 340], BF16)
        nc.scalar.copy(tmpb[:], ps_c1[:])
        nc.vector.tensor_tensor(accd[:], ps_c0[:], tmpb[:], OP.add)
        nc.scalar.activation(x2p[:], accd[:], AF.Silu, bias=b_dwp)
        for g in range(4):
            nc.sync.dma_start(
                x12_3[32:64, 2 + 5 * g : 7 + 5 * g, :],
                x2p[32 * g : 32 * g + 32, :].rearrange(
                    "p (r c) -> p r c", c=XW
                )[:, 0:5, 0:W],
            )

        # ---- comp px: 1x1 conv 64->64 (+ SiLU)
        for r0, nr in ((0, 8), (8, 8), (16, 4)):
            ps = psA.tile([64, 512], F32, tag="convps")
            nc.tensor.matmul(
                ps[:, : nr * W], w_px,
                x12[0:64, (2 + r0) * W : (2 + r0 + nr) * W],
                start=True, stop=True,
            )
            nc.scalar.activation(
                enc_in[0:64, r0 * W : (r0 + nr) * W], ps[:, : nr * W],
                AF.Silu, bias=b_px,
            )

        # ---- enc cv1: 1x1 conv 64->50 (+ SiLU, mask row rides K=65)
        for r0, nr in ((0, 8), (8, 8), (16, 4)):
            ps = psA.tile([50, 512], F32, tag="convps")
            nc.tensor.matmul(
                ps[:, : nr * W], w_ecv1,
                enc_in[0:65, r0 * W : (r0 + nr) * W],
                start=True, stop=True,
            )
            nc.scalar.activation(
                e1c[0:50, r0 * W : (r0 + nr) * W], ps[:, : nr * W],
                AF.Silu, bias=b_ecv1,
            )

        # ---- enc dw3/dw5: 25 diag matmuls, 2 PSUM chunks of 272
        for g in range(2):
            nc.sync.dma_start(
                e1p3[50 * g : 50 * g + 50, 0:12, 2 : 2 + W],
                e1c3[0:50, 8 * g : 8 * g + 12, :],
            )
        nc.sync.dma_start(enc_cat[0:50, :], e1c[0:50, 2 * W : 18 * W])
        # 2-bank rotation (chunk0/chunk1): LDW pipelines with streaming
        ps_e0 = psDW.tile([100, 272], F32, tag="dwe0")
        ps_e1 = psDW.tile([100, 272], F32, tag="dwe1")
        for t in range(25):
            ty, tx = divmod(t, 5)
            off = ty * XW + tx
            lhsT = diage[0:100, 100 * t : 100 * t + 100]
            nc.tensor.matmul(ps_e0[:], lhsT, e1p[0:100, off : off + 272],
                             start=(t == 0), stop=(t == 24))
            nc.tensor.matmul(ps_e1[:], lhsT, e1p[0:100, off + 272 : off + 544],
                             start=(t == 0), stop=(t == 24))
        nc.scalar.activation(e2p[:, 0:272], ps_e0[:], AF.Silu, bias=b_edwp)
        nc.scalar.activation(e2p[:, 272:544], ps_e1[:], AF.Silu, bias=b_edwp)
        for g in range(2):
            nc.sync.dma_start(
                enc_cat3[50:100, 8 * g : 8 * g + 8, :],
                e2p[50 * g : 50 * g + 50, :].rearrange(
                    "p (r c) -> p r c", c=XW
                )[:, 0:8, 0:W],
            )

        psDW_cm.__exit__(None, None, None)
        psA_cm.__exit__(None, None, None)
        psE_cm = tc.tile_pool(name="psE", bufs=2, space="PSUM")
        psE = psE_cm.__enter__()

        # ---- enc px (transposed output: M = 128 pixels per row-pair)
        for t in range(8):
            ps = psE.tile([128, 100], F32, tag="encpx")
            nc.tensor.matmul(
                ps[:], enc_cat[0:101, 128 * t : 128 * t + 128],
                w_epx, start=True, stop=True,
            )
            nc.scalar.activation(ET[:, 100 * t : 100 * t + 100], ps[:], AF.Silu)

        psE_cm.__exit__(None, None, None)
        psT = ctx.enter_context(tc.tile_pool(name="psT", bufs=3, space="PSUM"))
        psO = ctx.enter_context(tc.tile_pool(name="psO", bufs=2, space="PSUM"))

        # ---- per row-pair: softmax -> scatter -> transpose -> matmul
        for t in range(8):
            te = expb[:, 100 * t : 100 * t + 100]
            tw = wcats[:, 100 * t : 100 * t + 100]
            # exp (bf16), sum over k (stride-4), 1/S
            nc.scalar.activation(te, ET3[:, t, :], AF.Exp)
            nc.vector.tensor_reduce(
                Sv[:, t], te.rearrange("p (k s) -> p s k", s=4),
                mybir.AxisListType.X, OP.add,
            )
            nc.vector.reciprocal(Rv[:, t], Sv[:, t])
            # scale: exp * R, R broadcast over k
            rb = Rv[:, t].unsqueeze(1).broadcast_to([128, 25, 4])
            nc.vector.tensor_tensor(
                tw.rearrange("p (k s) -> p k s", s=4),
                te.rearrange("p (k s) -> p k s", s=4), rb, OP.mult,
            )
            # pixel-space band scatter
            bx = b4x[:, 480 * t : 480 * t + 480]
            nc.gpsimd.local_scatter(
                bx, tw, sidx[:], channels=128, num_elems=480, num_idxs=100,
            )
            # 4 plane transposes -> interleaved b4all
            b4all = spool.tile([120, 512], BF16, tag="b4all")
            bview = b4all[:].rearrange(
                "p (jb yl dy xl dx) -> p dy dx yl jb xl",
                jb=4, yl=2, dy=2, xl=16, dx=2,
            )
            for s in range(4):
                pst = psT.tile([120, 128], BF16, tag="tr")
                nc.tensor.transpose(pst[:], bx[:, 120 * s : 120 * s + 120],
                                    ident)
                src = pst[:].rearrange("p (yl jb xl) -> p yl jb xl",
                                       yl=2, jb=4)
                dst = bview[:, s // 2, s % 2]
                if s % 2 == 0:
                    nc.vector.tensor_copy(dst, src)
                else:
                    nc.scalar.copy(dst, src)
            # 4 output matmuls into one PSUM bank, written pre-interleaved
            # (block jb's (r, j) columns land at psum col r*128 + 32*jb + j)
            po = psO.tile([128, 512], F32, tag="out")
            po3 = po[:].rearrange("c (r j) -> c r j", j=128)
            for jb in range(4):
                nc.tensor.matmul(
                    po3[:, :, 32 * jb : 32 * jb + 32],
                    xt[:, 512 * t + 128 * jb : 512 * t + 128 * jb + 128],
                    b4all[:, 128 * jb : 128 * jb + 128],
                    start=True, stop=True,
                )
            stg = spool.tile([128, 512], F32, tag="ostage")
            nc.vector.tensor_copy(stg[:, 0:256], po[:, 0:256])
            nc.scalar.copy(stg[:, 256:512], po[:, 256:512])
            (nc.sync if t % 2 == 0 else nc.scalar).dma_start(
                out3[:, 4 * t : 4 * t + 4, :],
                stg[:].rearrange("c (r j) -> c r j", j=128),
            )

    nc.compile()
    return nc


_NC_CACHE = None


def _get_nc():
    global _NC_CACHE
    if _NC_CACHE is None:
        _NC_CACHE = build_kernel()
    return _NC_CACHE


def kernel(**inputs) -> np.ndarray:
    X = np.asarray(inputs["X"], np.float32)
    consts = _host_consts(
        {k: np.asarray(v, np.float32) for k, v in inputs.items() if k != "X"}
    )
    in_maps = []
    for core in range(NCORES):
        xs, mrow, emask, xt = _host_shard(X, core)
        m = dict(consts)
        m["x"] = xs
        m["mrow"] = mrow
        m["emask"] = emask
        m["xt"] = xt
        in_maps.append(m)

    nc = _get_nc()
    res = run_bass_kernel_spmd(nc, in_maps, core_ids=list(range(NCORES)))
    out = np.zeros((2, C, 128, 128), np.float32)
    for core in range(NCORES):
        b, ri = divmod(core, 4)
        out[b, :, 32 * ri : 32 * ri + 32, :] = (
            res.results[core]["out"].reshape(C, 32, 128)
        )
    return out


if __name__ == "__main__":
    print("smoke build only")
    build_kernel()
    print("build ok")


# revision 47
# speedup vs baseline: 1.3055x; 1.0580x over previous
"""CARAFE + MSGConv Trainium2 kernel (8 NeuronCores, spatial x batch sharding).

out[c, i, j] = sum_{p,q} W[5p+q, i, j] * Xpad[c, i//2 + p - 2, j//2 + q - 2]
 (CARAFE taps live at source resolution; identical for both subpixel parities).

Per core: one batch element (core//4) and a 16-source-row block (core%4).

v2 design:
- Depthwise convs run on the TensorEngine as 25 accumulating diagonal
  matmuls per conv (lhsT = diag(w[:,t]) shipped from host), alternating
  two PSUM banks so weight loads pipeline with streaming.
- The 25-tap reassembly weights are scattered in PIXEL space (100 idx per
  row-pair instead of 400 in output space: no quadrant replication), then
  4 per-subpixel-plane PE transposes write an interleaved SBUF layout so
  each output block's matmul rhs is fully contiguous.
- Softmax runs per row-pair (exp -> strided reduce -> recip -> one
  broadcast tensor_tensor), pipelined with the scatter/transpose/matmul
  back end.
"""

import sys

sys.path.insert(0, "/opt/trn_rl_repo")

from contextlib import ExitStack

import ml_dtypes
import numpy as np

import concourse.bass as bass
import concourse.tile as tile
from concourse import bacc, library_config, mybir
from concourse.bass_utils import run_bass_kernel_spmd

BF16 = mybir.dt.bfloat16
F32 = mybir.dt.float32
I16 = mybir.dt.int16
AF = mybir.ActivationFunctionType
OP = mybir.AluOpType
nbf = ml_dtypes.bfloat16

C = 128
H = W = 64
NCORES = 8
XR = 24          # X shard rows (16 + 4 halo each side)
XW = 68          # padded width for dw slabs only
NEG = -30.0      # additive pre-activation mask; SiLU(-30) ~= -2.8e-12


# ======================================================================
# host-side parameter prep
# ======================================================================

def _fold_1x1(w, s):
    return (w[:, :, 0, 0] * s[:, None]).T.copy()


def _dw_taps(w, s, k):
    ch = w.shape[0]
    out = np.zeros((ch, 25), np.float32)
    off = (5 - k) // 2
    for ty in range(k):
        for tx in range(k):
            out[:, 5 * (ty + off) + (tx + off)] = w[:, 0, ty, tx] * s
    return out


def _host_consts(inputs):
    d = {}
    w_cv1 = _fold_1x1(inputs["comp_cv1_w"], inputs["comp_cv1_s"])
    b_cv1 = inputs["comp_cv1_b"]
    w3 = _dw_taps(inputs["comp_dw3_w"], inputs["comp_dw3_s"], 3)
    w5 = _dw_taps(inputs["comp_dw5_w"], inputs["comp_dw5_s"], 5)
    w_dwp = np.tile(np.concatenate([w3, w5], 0), (4, 1))      # [128, 25]
    b_dwp = np.tile(
        np.concatenate([inputs["comp_dw3_b"], inputs["comp_dw5_b"]]), 4
    )
    w_px = _fold_1x1(inputs["comp_px_w"], inputs["comp_px_s"])
    b_px = inputs["comp_px_b"]
    we = _fold_1x1(inputs["enc_cv1_w"], inputs["enc_cv1_s"])
    w_ecv1 = np.concatenate([we, np.ones((1, 50), np.float32)], 0)
    b_ecv1 = inputs["enc_cv1_b"]
    e3 = _dw_taps(inputs["enc_dw3_w"], inputs["enc_dw3_s"], 3)
    e5 = _dw_taps(inputs["enc_dw5_w"], inputs["enc_dw5_s"], 5)
    w_edwp = np.tile(np.concatenate([e3, e5], 0), (2, 1))     # [100, 25]
    b_edwp = np.tile(
        np.concatenate([inputs["enc_dw3_b"], inputs["enc_dw5_b"]]), 2
    )
    wpx = _fold_1x1(inputs["enc_px_w"], inputs["enc_px_s"])
    w_epx = np.concatenate([wpx, inputs["enc_px_b"].reshape(1, 100)], 0)

    # packa bf16 [128, 630]: w_cv1 | w_px | w_ecv1 | w_epx | ident |
    #   w_px group replicas: K=64 blocks with the inactive 32-row half
    #   zeroed, for base-0/base-64 matmuls over the packed group layout
    pa = np.zeros((128, 630), np.float32)
    pa[0:128, 0:32] = w_cv1
    pa[0:64, 32:96] = w_px
    pa[0:65, 96:146] = w_ecv1
    pa[0:101, 146:246] = w_epx
    pa[0:128, 246:374] = np.eye(128)
    for half in range(2):
        for h in range(2):
            rows = slice(64 * half + 32 * h, 64 * half + 32 * h + 32)
            pa[rows, 374 + 64 * h : 438 + 64 * h] = w_px[0:32]
            pa[rows, 502 + 64 * h : 566 + 64 * h] = w_px[32:64]
    d["packa"] = pa.astype(nbf)
    # packb f32 [128, 5]: biases only
    pb = np.zeros((128, 5), np.float32)
    pb[:, 0] = b_dwp
    pb[0:100, 1] = b_edwp
    pb[0:32, 2] = b_cv1
    pb[0:64, 3] = b_px
    pb[0:50, 4] = b_ecv1
    d["packb"] = pb

    # depthwise tap weights (diag matrices are built on-device by scatter);
    # chunk layout: 12 taps + 14 taps (num_idxs must be even; the 14th
    # duplicates tap 24, writing the same diag slot twice)
    wt = np.zeros((128, 52), np.float32)
    wt[:, 0:12] = w_dwp[:, 0:12]
    wt[:, 12:25] = w_dwp[:, 12:25]
    wt[:, 25] = w_dwp[:, 24]
    wt[0:100, 26:38] = w_edwp[:, 0:12]
    wt[0:100, 38:51] = w_edwp[:, 12:25]
    wt[0:100, 51] = w_edwp[:, 24]
    d["wtap"] = wt.astype(nbf)
    # scatter indices for the diag build: col j -> diag block j, own row
    dg = np.zeros((128, 28), np.int16)
    for p in range(128):
        for j in range(14):
            dg[p, j] = 128 * min(j, 12) + p
            dg[p, 14 + j] = 100 * min(j, 12) + p if p < 100 else 0
    d["dgix"] = dg

    d["ones1"] = np.ones((1, 32), nbf)
    d["erow1"] = np.ones((1, 16 * W), nbf)

    # sidx [128, 100] int16: pixel-space scatter, pix = 64*yl + x
    # entry e = 4k+s -> 120*s + 20*(yl+p) + (x%16 + q), k = 5p+q
    si = np.zeros((128, 100), np.int16)
    for pix in range(128):
        yl, x = divmod(pix, 64)
        xl = x % 16
        for k in range(25):
            p, q = divmod(k, 5)
            for s in range(4):
                si[pix, 4 * k + s] = 120 * s + 20 * (yl + p) + (xl + q)
    d["sidx"] = si
    return d


def _host_shard(X, core):
    b, ri = divmod(core, 4)
    r0 = 16 * ri - 4
    xs = np.zeros((C, XR, W), np.float32)
    lo, hi = max(0, r0), min(H, r0 + XR)
    xs[:, lo - r0 : hi - r0, :] = X[b, :, lo:hi, :]
    mrow = np.zeros((1, XR, W), np.float32)
    for r in range(XR):
        if not (0 <= r0 + r < H):
            mrow[0, r, :] = NEG
    emask = np.zeros((1, 20, W), np.float32)
    for r in range(20):
        if not (0 <= (16 * ri - 2) + r < H):
            emask[0, r, :] = NEG
    xsb = xs.astype(nbf)
    # pre-transposed X slabs, one [120, 128] per block (column-padded)
    xsp = np.zeros((C, XR, XW), nbf)
    xsp[:, :, 2 : 2 + W] = xsb
    xt = np.zeros((120, 32 * 128), nbf)
    for B in range(32):
        t, jb = divmod(B, 4)
        slab = xsp[:, 2 * t + 2 : 2 * t + 8, 16 * jb : 16 * jb + 20]
        xt[:, 128 * B : 128 * B + 128] = slab.reshape(C, 120).T
    return (
        xsb.reshape(C, XR * W),
        mrow.reshape(1, XR * W).astype(nbf),
        emask.reshape(1, 20 * W).astype(nbf),
        xt,
    )


# ======================================================================
# device kernel
# ======================================================================

def build_kernel():
    nc = bacc.Bacc(
        "TRN2",
        target_bir_lowering=False,
        debug=False,
        enable_asserts=False,
        num_devices=NCORES,
    )

    def din(name, shape, dt):
        return nc.dram_tensor(name, list(shape), dt, kind="ExternalInput").ap()

    x_d = din("x", (128, XR * W), BF16)
    xt_d = din("xt", (120, 32 * 128), BF16)
    mrow_d = din("mrow", (1, XR * W), BF16)
    emask_d = din("emask", (1, 20 * W), BF16)
    erow1_d = din("erow1", (1, 16 * W), BF16)
    ones1_d = din("ones1", (1, 32), BF16)
    packa_d = din("packa", (128, 630), BF16)
    packb_d = din("packb", (128, 5), F32)
    wtap_d = din("wtap", (128, 52), BF16)
    dgix_d = din("dgix", (128, 28), I16)
    sidx_d = din("sidx", (128, 100), I16)
    out_d = nc.dram_tensor("out", [128, 32 * 128], F32, kind="ExternalOutput").ap()
    out3 = out_d.rearrange("c (r j) -> c r j", j=128)

    with tile.TileContext(nc) as tc, ExitStack() as ctx:
        cpool = ctx.enter_context(tc.tile_pool(name="consts", bufs=1))
        work = ctx.enter_context(tc.tile_pool(name="work", bufs=1))
        spool = ctx.enter_context(tc.tile_pool(name="stage", bufs=2))
        psA_cm = tc.tile_pool(name="psA", bufs=2, space="PSUM")
        psA = psA_cm.__enter__()
        psDW_cm = tc.tile_pool(name="psDW", bufs=1, space="PSUM")
        psDW = psDW_cm.__enter__()

        nc.gpsimd.load_library(library_config.local_scatter)

        def cload(ap_d, shape, dt, eng=None):
            t = cpool.tile(list(shape), dt, tag=ap_d.tensor.name)
            (eng or nc.sync).dma_start(t[:], ap_d)
            return t

        # sync queue: cv1 weights first, then the X chunks, then the rest
        packa = cpool.tile([128, 630], BF16, tag="packa")
        nc.sync.dma_start(packa[:, 0:32], packa_d[:, 0:32])
        packb = cload(packb_d, (128, 5), F32, eng=nc.scalar)
        mrow = cload(mrow_d, (1, XR * W), BF16, eng=nc.scalar)
        ones1 = cload(ones1_d, (1, 32), BF16, eng=nc.scalar)
        w_cv1 = packa[0:128, 0:32]
        w_px = packa[0:64, 32:96]
        w_ecv1 = packa[0:65, 96:146]
        w_epx = packa[0:101, 146:246]
        ident = packa[0:128, 246:374]
        b_dwp = packb[0:128, 0:1]
        b_edwp = packb[0:100, 1:2]
        b_cv1 = packb[0:32, 2:3]
        b_px = packb[0:64, 3:4]
        b_ecv1 = packb[0:50, 4:5]
        xb = cpool.tile([128, XR * W], BF16, tag="x")
        wtap = cload(wtap_d, (128, 52), BF16, eng=nc.gpsimd)
        dgix = cload(dgix_d, (128, 28), I16, eng=nc.gpsimd)
        for ch in range(3):
            (nc.sync if ch != 1 else nc.gpsimd).dma_start(
                xb[:, 8 * W * ch : 8 * W * (ch + 1)],
                x_d[:, 8 * W * ch : 8 * W * (ch + 1)],
            )
        nc.sync.dma_start(packa[:, 32:630], packa_d[:, 32:630])
        sidx = cload(sidx_d, (128, 100), I16, eng=nc.gpsimd)
        xt = cpool.tile([120, 32 * 128], BF16, tag="xt")
        for ch in range(2):
            nc.gpsimd.dma_start(
                xt[:, 2048 * ch : 2048 * (ch + 1)],
                xt_d[:, 2048 * ch : 2048 * (ch + 1)],
            )

        # warmup: trigger the local_scatter ucode library load early
        warm = work.tile([16, 16], BF16)
        nc.gpsimd.local_scatter(
            warm[:], packa[0:16, 0:2], dgix[:][0:16, 0:2],
            channels=16, num_elems=16, num_idxs=2,
        )

        # build the depthwise diag matrices on-device (scatter zero-fills)
        diagc = cpool.tile([128, 3200], BF16, tag="diagc")
        diage = cpool.tile([128, 2500], BF16, tag="diage")
        nc.gpsimd.local_scatter(
            diagc[:, 0:1536], wtap[:, 0:12], dgix[:, 0:12],
            channels=128, num_elems=1536, num_idxs=12,
        )
        nc.gpsimd.local_scatter(
            diagc[:, 1536:3200], wtap[:, 12:26], dgix[:, 0:14],
            channels=128, num_elems=1664, num_idxs=14,
        )
        nc.gpsimd.local_scatter(
            diage[:, 0:1200], wtap[:, 26:38], dgix[:, 14:26],
            channels=128, num_elems=1200, num_idxs=12,
        )
        nc.gpsimd.local_scatter(
            diage[:, 1200:2500], wtap[:, 38:52], dgix[:, 14:28],
            channels=128, num_elems=1300, num_idxs=14,
        )

        # preload the EXP act table before any SILU runs (the mid-kernel
        # SILU->EXP table switch otherwise stalls the softmax by ~1.3us)
        dume = work.tile([1, 4], BF16)
        nc.scalar.activation(dume[:], packb[0:1, 0:4], AF.Exp)

        # persistent working tensors
        x12 = work.tile([32, XR * W], BF16)        # x1 only
        enc_in = work.tile([65, 20 * W], BF16)     # px out + mask row
        enc_cat = work.tile([101, 16 * W], BF16)   # enc x1/x2 + ones row
        e1c = work.tile([50, 20 * W], BF16)        # enc cv1 out
        x1p = work.tile([128, 9 * XW + 8], BF16)   # packed x1 (68-pitch)
        e1p = work.tile([100, 12 * XW + 8], BF16)  # packed enc x1 (68-pitch)
        ET = work.tile([128, 800], F32)            # enc px logits
        expb = work.tile([128, 800], BF16)         # exp values [t][4k+s]
        S = work.tile([128, 32], F32)
        R = work.tile([128, 32], F32)
        wcats = work.tile([128, 800], BF16)        # softmaxed weights
        b4x = work.tile([128, 8 * 480], BF16)      # pix-space bands
        x2p = work.tile([128, 340], BF16)
        e2p = work.tile([100, 544], BF16)

        xb3 = xb[:].rearrange("p (r c) -> p r c", c=W)
        x12_3 = x12[:].rearrange("p (r c) -> p r c", c=W)
        enc_cat3 = enc_cat[:].rearrange("p (r c) -> p r c", c=W)
        e1c3 = e1c[:].rearrange("p (r c) -> p r c", c=W)
        x1p3 = x1p[:, 0 : 9 * XW].rearrange("p (r c) -> p r c", c=XW)
        e1p3 = e1p[:, 0 : 12 * XW].rearrange("p (r c) -> p r c", c=XW)
        ET3 = ET[:].rearrange("p (t e) -> p t e", e=100)
        Rv = R[:].rearrange("p (t s) -> p t s", s=4)
        Sv = S[:].rearrange("p (t s) -> p t s", s=4)

        # zero the dw-slab pad columns (cols 0:2 and 66:68)
        nc.vector.memset(x1p[:, 9 * XW : 9 * XW + 8], 0.0)
        nc.vector.memset(e1p[:, 12 * XW : 12 * XW + 8], 0.0)
        nc.vector.memset(x1p3[:, :, 0:2], 0.0)
        nc.vector.memset(x1p3[:, :, 66:68], 0.0)
        nc.vector.memset(e1p3[:, :, 0:2], 0.0)
        nc.vector.memset(e1p3[:, :, 66:68], 0.0)
        nc.sync.dma_start(enc_cat[100:101, :], erow1_d)
        nc.sync.dma_start(enc_in[64:65, :], emask_d)

        # ---- comp cv1: 1x1 conv 128->32 (+ SiLU + out-of-image row mask)
        for ch in range(3):
            ps = psA.tile([32, 512], F32, tag="convps")
            nc.tensor.matmul(
                ps[:], w_cv1, xb[:, 512 * ch : 512 * (ch + 1)],
                start=True, stop=False,
            )
            nc.tensor.matmul(
                ps[:], ones1[:], mrow[:, 512 * ch : 512 * (ch + 1)],
                start=False, stop=True,
            )
            nc.scalar.activation(
                x12[0:32, 512 * ch : 512 * (ch + 1)], ps[:],
                AF.Silu, bias=b_cv1,
            )

        # ---- comp dw3/dw5: 25 diag matmuls, 2 PSUM chunks of 170
        for g in range(4):
            nc.sync.dma_start(
                x1p3[32 * g : 32 * g + 32, 0:9, 2 : 2 + W],
                x12_3[0:32, 5 * g : 5 * g + 9, :],
            )
        # even taps accumulate in bank A, odd in bank B (pipelined LDW)
        ps_c0 = psDW.tile([128, 340], F32, tag="dwc0")
        ps_c1 = psDW.tile([128, 340], F32, tag="dwc1")
        for t in range(25):
            ty, tx = divmod(t, 5)
            off = ty * XW + tx
            nc.tensor.matmul((ps_c0 if t % 2 == 0 else ps_c1)[:],
                             diagc[:, 128 * t : 128 * t + 128],
                             x1p[:, off : off + 340],
                             start=(t < 2), stop=(t >= 23))
        tmpb = work.tile([128, 340], BF16)
        accd = work.tile([128, 340], BF16)
        nc.scalar.copy(tmpb[:], ps_c1[:])
        nc.vector.tensor_tensor(accd[:], ps_c0[:], tmpb[:], OP.add)
        nc.scalar.activation(x2p[:], accd[:], AF.Silu, bias=b_dwp)

        # ---- comp px: 1x1 conv 64->64 (+ SiLU), reading the packed
        # group layouts directly (x1 from x1p rows 2:7, x2 from x2p)
        x2p3 = x2p[:].rearrange("p (r c) -> p r c", c=XW)
        for g in range(4):
            b, h = 64 * (g // 2), g % 2
            ps = psA.tile([64, 512], F32, tag="convps")
            nc.tensor.matmul(
                ps[:, 0:320], packa[b : b + 64, 374 + 64 * h : 438 + 64 * h],
                x1p3[b : b + 64, 2:7, 2 : 2 + W],
                start=True, stop=False,
            )
            nc.tensor.matmul(
                ps[:, 0:320], packa[b : b + 64, 502 + 64 * h : 566 + 64 * h],
                x2p3[b : b + 64, 0:5, 0:W],
                start=False, stop=True,
            )
            nc.scalar.activation(
                enc_in[0:64, 320 * g : 320 * (g + 1)], ps[:, 0:320],
                AF.Silu, bias=b_px,
            )

        # ---- enc cv1: 1x1 conv 64->50 (+ SiLU, mask row rides K=65)
        for r0, nr in ((0, 8), (8, 8), (16, 4)):
            ps = psA.tile([50, 512], F32, tag="convps")
            nc.tensor.matmul(
                ps[:, : nr * W], w_ecv1,
                enc_in[0:65, r0 * W : (r0 + nr) * W],
                start=True, stop=True,
            )
            nc.scalar.activation(
                e1c[0:50, r0 * W : (r0 + nr) * W], ps[:, : nr * W],
                AF.Silu, bias=b_ecv1,
            )

        # ---- enc dw3/dw5: 25 diag matmuls, 2 PSUM chunks of 272
        for g in range(2):
            nc.sync.dma_start(
                e1p3[50 * g : 50 * g + 50, 0:12, 2 : 2 + W],
                e1c3[0:50, 8 * g : 8 * g + 12, :],
            )
        nc.sync.dma_start(enc_cat[0:50, :], e1c[0:50, 2 * W : 18 * W])
        # 2-bank rotation (chunk0/chunk1): LDW pipelines with streaming
        ps_e0 = psDW.tile([100, 272], F32, tag="dwe0")
        ps_e1 = psDW.tile([100, 272], F32, tag="dwe1")
        for t in range(25):
            ty, tx = divmod(t, 5)
            off = ty * XW + tx
            lhsT = diage[0:100, 100 * t : 100 * t + 100]
            nc.tensor.matmul(ps_e0[:], lhsT, e1p[0:100, off : off + 272],
                             start=(t == 0), stop=(t == 24))
            nc.tensor.matmul(ps_e1[:], lhsT, e1p[0:100, off + 272 : off + 544],
                             start=(t == 0), stop=(t == 24))
        nc.scalar.activation(e2p[:, 0:272], ps_e0[:], AF.Silu, bias=b_edwp)
        nc.scalar.activation(e2p[:, 272:544], ps_e1[:], AF.Silu, bias=b_edwp)
        for g in range(2):
            nc.sync.dma_start(
                enc_cat3[50:100, 8 * g : 8 * g + 8, :],
                e2p[50 * g : 50 * g + 50, :].rearrange(
                    "p (r c) -> p r c", c=XW
                )[:, 0:8, 0:W],
            )

        psDW_cm.__exit__(None, None, None)
        psA_cm.__exit__(None, None, None)
        psE_cm = tc.tile_pool(name="psE", bufs=2, space="PSUM")
        psE = psE_cm.__enter__()

        # ---- enc px (transposed output: M = 128 pixels per row-pair)
        for t in range(8):
            ps = psE.tile([128, 100], F32, tag="encpx")
            nc.tensor.matmul(
                ps[:], enc_cat[0:101, 128 * t : 128 * t + 128],
                w_epx, start=True, stop=True,
            )
            nc.scalar.activation(ET[:, 100 * t : 100 * t + 100], ps[:], AF.Silu)

        psE_cm.__exit__(None, None, None)
        psT = ctx.enter_context(tc.tile_pool(name="psT", bufs=3, space="PSUM"))
        psO = ctx.enter_context(tc.tile_pool(name="psO", bufs=2, space="PSUM"))

        # ---- per row-pair: softmax -> scatter -> transpose -> matmul
        for t in range(8):
            te = expb[:, 100 * t : 100 * t + 100]
            tw = wcats[:, 100 * t : 100 * t + 100]
            # exp (bf16), sum over k (stride-4), 1/S
            nc.scalar.activation(te, ET3[:, t, :], AF.Exp)
            nc.vector.tensor_reduce(
                Sv[:, t], te.rearrange("p (k s) -> p s k", s=4),
                mybir.AxisListType.X, OP.add,
            )
            nc.vector.reciprocal(Rv[:, t], Sv[:, t])
            # scale: exp * R, R broadcast over k
            rb = Rv[:, t].unsqueeze(1).broadcast_to([128, 25, 4])
            nc.vector.tensor_tensor(
                tw.rearrange("p (k s) -> p k s", s=4),
                te.rearrange("p (k s) -> p k s", s=4), rb, OP.mult,
            )
            # pixel-space band scatter
            bx = b4x[:, 480 * t : 480 * t + 480]
            nc.gpsimd.local_scatter(
                bx, tw, sidx[:], channels=128, num_elems=480, num_idxs=100,
            )
            # 4 plane transposes -> interleaved b4all
            b4all = spool.tile([120, 512], BF16, tag="b4all")
            bview = b4all[:].rearrange(
                "p (jb yl dy xl dx) -> p dy dx yl jb xl",
                jb=4, yl=2, dy=2, xl=16, dx=2,
            )
            for s in range(4):
                pst = psT.tile([120, 128], BF16, tag="tr")
                nc.tensor.transpose(pst[:], bx[:, 120 * s : 120 * s + 120],
                                    ident)
                src = pst[:].rearrange("p (yl jb xl) -> p yl jb xl",
                                       yl=2, jb=4)
                dst = bview[:, s // 2, s % 2]
                if s % 2 == 0:
                    nc.vector.tensor_copy(dst, src)
                else:
                    nc.scalar.copy(dst, src)
            # 4 output matmuls into one PSUM bank, written pre-interleaved
            # (block jb's (r, j) columns land at psum col r*128 + 32*jb + j)
            po = psO.tile([128, 512], F32, tag="out")
            po3 = po[:].rearrange("c (r j) -> c r j", j=128)
            for jb in range(4):
                nc.tensor.matmul(
                    po3[:, :, 32 * jb : 32 * jb + 32],
                    xt[:, 512 * t + 128 * jb : 512 * t + 128 * jb + 128],
                    b4all[:, 128 * jb : 128 * jb + 128],
                    start=True, stop=True,
                )
            stg = spool.tile([128, 512], F32, tag="ostage")
            nc.vector.tensor_copy(stg[:, 0:256], po[:, 0:256])
            nc.scalar.copy(stg[:, 256:512], po[:, 256:512])
            (nc.sync if t % 2 == 0 else nc.scalar).dma_start(
                out3[:, 4 * t : 4 * t + 4, :],
                stg[:].rearrange("c (r j) -> c r j", j=128),
            )

    nc.compile()
    return nc


_NC_CACHE = None


def _get_nc():
    global _NC_CACHE
    if _NC_CACHE is None:
        _NC_CACHE = build_kernel()
    return _NC_CACHE


def kernel(**inputs) -> np.ndarray:
    X = np.asarray(inputs["X"], np.float32)
    consts = _host_consts(
        {k: np.asarray(v, np.float32) for k, v in inputs.items() if k != "X"}
    )
    in_maps = []
    for core in range(NCORES):
        xs, mrow, emask, xt = _host_shard(X, core)
        m = dict(consts)
        m["x"] = xs
        m["mrow"] = mrow
        m["emask"] = emask
        m["xt"] = xt
        in_maps.append(m)

    nc = _get_nc()
    res = run_bass_kernel_spmd(nc, in_maps, core_ids=list(range(NCORES)))
    out = np.zeros((2, C, 128, 128), np.float32)
    for core in range(NCORES):
        b, ri = divmod(core, 4)
        out[b, :, 32 * ri : 32 * ri + 32, :] = (
            res.results[core]["out"].reshape(C, 32, 128)
        )
    return out


if __name__ == "__main__":
    print("smoke build only")
    build_kernel()
    print("build ok")
